# revision 42
# baseline (speedup 1.0000x reference)
"""Trainium2 Bass kernel for nn_GunnarODE: neural CDE with hermite spline control.

Contract: kernel(**inputs) takes FULL unsharded inputs (ts, us, ys, W1, b1,
W2, b2, batch_size) and returns the FULL (B, L, Y) output. Internally shards
the batch across 8 NeuronCores (pure data parallel), runs a Bass/Tile kernel
per core, and reassembles.

Structure (v4 — see kernel_baseline.py for the original):
  - x = concat([t, us]) with unit-spaced knots => dt == 1; slopes are
    replicated host-side to the 128 vfc rows and streamed as ONE
    (NI, 128, B) array (1 tile/interval — more streams choke the DMA
    descriptor path, measured). dXb_i = s_{k-1} + beta_i*(s_k - s_{k-1})
    (beta = [0,.8125,1.25,1.3125]): D one interval ahead on GpSimd, three
    batched stt combines per interval on DVE, all off the critical path.
  - State is hpre = W1 @ z held in PSUM (accumulated across all substeps).
    Critical path per substep (half-split columns for pipelining):
    tanh(th) -> W2a matmul -> tanh -> mult/add -> W1Sel matmul.
  - Time channel: vft is computed replicated to all 128 rows (W2bRep; same
    PE cost since matmul cost ~ N) into the second half of each per-half
    PSUM tile [vfc_h | vpr_h]; its contribution enters hpre via a second
    accumulating matmul with W1Sel/8 (== 0.25*W1@vft exactly), placed in
    the PE gap while tmp cooks. Keeping the PE dense is critical: PE
    idling makes the HAM clock-gate re-throttle 2.4->1.2 GHz and doubles
    every matmul (measured, catastrophic).
  - PSUM/engine discipline (all measured, each worth 2x+): per-half PSUM
    tiles (deps are tile-granular; shared tiles serialize the half-chains),
    per-stage paired emission (engines are strict FIFO), one input stream
    (DMA descriptor flood collapses throughput).
  - z is reconstructed once per interval via RT = pinv(W1) in float32r
    (quarter-cost matmul): recon error is output-only, never re-enters the
    state, so tf32-class precision is fine there.
  - Everything else fp32: the ODE is chaotic (~1e5 error amplification);
    bf16/tf32-class matmuls in the state loop measurably fail the 2e-2
    budget (CPU-emulated: bf16 -> 0.31, tf32 -> 0.16 rel err).
"""
import sys
if '/opt/trn_rl_repo' not in sys.path:
    sys.path.insert(0, '/opt/trn_rl_repo')

import numpy as np

N_CORES = 8
L = 512
B_TOT = 4096
U = 8
Y = 16
H = 128
NI = L - 1            # intervals
HSTEP = 0.25          # dt / SUBSTEPS with dt == 1
B_LOC = B_TOT // N_CORES  # 512
HB = B_LOC // 2

BETA = [0.0, 0.8125, 1.25, 1.3125]

_BUILD_CACHE = {}


def _host_constants(W1, b1, W2, b2):
    """Precompute transposed/permuted constant matrices (host-side, free)."""
    # vfc row r <-> W2 output row (y = r%16, channel c = r//16 + 1)
    rowmap = np.array([(r % 16) * 9 + (r // 16 + 1) for r in range(128)])
    cst = {}
    cst["W1T"] = np.ascontiguousarray(W1.T)                        # (16,128)
    cst["W2aT"] = np.ascontiguousarray(W2[rowmap, :].T)            # (128,128)
    # replicated time-channel weights: output row m holds vft[m%16]
    W2bT = np.ascontiguousarray(W2[np.arange(16) * 9, :].T)        # (128,16)
    cst["W2bRepT"] = np.ascontiguousarray(W2bT[:, [m % 16 for m in range(128)]])
    cst["b1c"] = np.ascontiguousarray(b1[:, None])                 # (128,1)
    cst["b2c"] = np.ascontiguousarray(b2[rowmap][:, None])         # (128,1)
    b2t = b2[np.arange(16) * 9]
    cst["b2tRep"] = np.ascontiguousarray(b2t[[m % 16 for m in range(128)]][:, None])
    # hpre state update: hpre += (h*W1*Sel^T) @ tmp
    # tmp = svfc * dXb + svft_rep/8  => through W1Sel this contributes
    # 0.25*W1@(Sel@(svfc*dXb)) + 0.25*W1@vft (each y-group sums 8 copies).
    w1selt = np.zeros((128, 128), dtype=np.float32)  # [r, j] = h*W1[j, r%16]
    for r in range(128):
        w1selt[r, :] = HSTEP * W1[:, r % 16]
    cst["W1SelT"] = w1selt
    # time-channel accumulate: hpre += (W1Sel/8) @ svt_rep == 0.25*W1@vft
    cst["W1SelT8"] = (w1selt / 8.0).astype(np.float32)
    # output reconstruction: z = pinv(W1) @ hpre  (W1 is 128x16, cond ~2)
    R = np.linalg.pinv(W1.astype(np.float64)).astype(np.float32)   # (16,128)
    cst["RT"] = np.ascontiguousarray(R.T)                          # (128,16)
    return {k: v.astype(np.float32) for k, v in cst.items()}


def _build(n_intervals, merged_tanh=False):
    """Build + compile the Bass module (cached per config)."""
    key = (n_intervals, merged_tanh)
    if key in _BUILD_CACHE:
        return _BUILD_CACHE[key]

    import concourse.bass as bass
    import concourse.bacc as bacc
    import concourse.tile as tile
    from concourse import mybir

    F32 = mybir.dt.float32
    F32R = mybir.dt.float32r
    TANH = mybir.ActivationFunctionType.Tanh
    MULT = mybir.AluOpType.mult
    ADD = mybir.AluOpType.add
    SUB = mybir.AluOpType.subtract

    nc = bacc.Bacc("TRN2", target_bir_lowering=False, debug=False,
                   num_devices=N_CORES)

    d_srep = nc.dram_tensor("srep", (n_intervals, 128, B_LOC), F32,
                            kind="ExternalInput")
    d_ys0 = nc.dram_tensor("ys0T", (16, B_LOC), F32, kind="ExternalInput")
    d_W1T = nc.dram_tensor("W1T", (16, 128), F32, kind="ExternalInput")
    d_W2aT = nc.dram_tensor("W2aT", (128, 128), F32, kind="ExternalInput")
    d_W2bRepT = nc.dram_tensor("W2bRepT", (128, 128), F32, kind="ExternalInput")
    d_b1 = nc.dram_tensor("b1c", (128, 1), F32, kind="ExternalInput")
    d_b2c = nc.dram_tensor("b2c", (128, 1), F32, kind="ExternalInput")
    d_b2tRep = nc.dram_tensor("b2tRep", (128, 1), F32, kind="ExternalInput")
    d_W1SelT = nc.dram_tensor("W1SelT", (128, 128), F32, kind="ExternalInput")
    d_W1SelT8 = nc.dram_tensor("W1SelT8", (128, 128), F32, kind="ExternalInput")
    d_RT = nc.dram_tensor("RT", (128, 16), F32, kind="ExternalInput")
    d_out = nc.dram_tensor("out", (n_intervals, 16, B_LOC), F32, kind="ExternalOutput")

    with tile.TileContext(nc) as tc:
        with (
            tc.tile_pool(name="consts", bufs=1) as consts,
            tc.tile_pool(name="zpool", bufs=2) as zpool,
            tc.tile_pool(name="work", bufs=2) as work,
            tc.tile_pool(name="srp", bufs=4) as srp,
            tc.tile_pool(name="dpool", bufs=2) as dpool,
            tc.tile_pool(name="dxp", bufs=2) as dxp,
            tc.tile_pool(name="ps1", bufs=1, space="PSUM") as ps1,
            tc.tile_pool(name="ps2", bufs=2, space="PSUM") as ps2,
            tc.tile_pool(name="ps3", bufs=1, space="PSUM") as ps3,
            tc.tile_pool(name="ps4", bufs=1, space="PSUM") as ps4,
        ):
            W1T = consts.tile([16, 128], F32)
            W2aT = consts.tile([128, 128], F32)
            W2bRepT = consts.tile([128, 128], F32)
            b1c = consts.tile([128, 1], F32)
            b2c = consts.tile([128, 1], F32)
            b2tRep = consts.tile([128, 1], F32)
            W1SelT = consts.tile([128, 128], F32)
            W1SelT8 = consts.tile([128, 128], F32)
            RT = consts.tile([128, 16], F32R)
            nc.sync.dma_start(W1T[:], d_W1T.ap())
            nc.sync.dma_start(W2aT[:], d_W2aT.ap())
            nc.sync.dma_start(W2bRepT[:], d_W2bRepT.ap())
            nc.sync.dma_start(b1c[:], d_b1.ap())
            nc.sync.dma_start(b2c[:], d_b2c.ap())
            nc.sync.dma_start(b2tRep[:], d_b2tRep.ap())
            nc.sync.dma_start(W1SelT[:], d_W1SelT.ap())
            nc.sync.dma_start(W1SelT8[:], d_W1SelT8.ap())
            nc.sync.dma_start(RT[:], d_RT.ap().bitcast(F32R))

            z0 = zpool.tile([16, B_LOC], F32, tag="z")
            nc.sync.dma_start(z0[:], d_ys0.ap())

            # hpre is THE state: a persistent PSUM accumulator holding W1 @ z.
            # Split per column-half: PSUM deps are tile-granular, a shared
            # tile serializes the two half-chains (measured +2us/substep).
            hpre = [ps1.tile([128, HB], F32, tag=f"hpre{h}", name=f"hpre{h}")
                    for h in range(2)]
            for h in range(2):
                nc.tensor.matmul(hpre[h][:], W1T[:], z0[:, h * HB:(h + 1) * HB],
                                 start=True, stop=False, skip_group_check=True)

            srs, Ds = {}, {}

            def load_srep(k):
                if k < n_intervals:
                    t = srp.tile([128, B_LOC], F32, tag="srep", name=f"srep_{k}")
                    nc.sync.dma_start(t[:], d_srep.ap()[k])
                    srs[k] = t

            def make_D(k):
                # D_k = s_k - s_{k-1}; for k=0 both are s_0 => zero tile
                if k < n_intervals:
                    D = dpool.tile([128, B_LOC], F32, tag="D", name=f"D_{k}")
                    prev = srs[k - 1] if k >= 1 else srs[0]
                    nc.gpsimd.tensor_tensor(D[:], srs[k][:], prev[:], SUB)
                    Ds[k] = D

            load_srep(0)
            load_srep(1)
            make_D(0)
            sprev = srs[0]  # s_{-1} := s_0 (backward-diff init)

            for k in range(n_intervals):
                load_srep(k + 2)
                make_D(k + 1)
                D = Ds.pop(k)
                for i in range(4):
                    # off-path dXb combine for this substep (runs in the DVE
                    # gap of the previous substep; i=0 uses sprev directly)
                    if i == 0:
                        dXb = sprev
                    else:
                        dXb = dxp.tile([128, B_LOC], F32, tag=f"dxb{i}",
                                       name=f"dxb_{k}_{i}")
                        nc.vector.scalar_tensor_tensor(
                            dXb[:], D[:], float(BETA[i]), sprev[:], MULT, ADD)
                    th = work.tile([128, B_LOC], F32, tag="th")
                    # per-half PSUM tiles for vfc; full-width tile for vpr
                    vv = [ps2.tile([128, HB], F32, tag=f"vv{h}",
                                   name=f"vv{h}_{k}_{i}") for h in range(2)]
                    vpr = ps3.tile([128, B_LOC], F32, tag="vpr",
                                   name=f"vpr_{k}_{i}")
                    svc = work.tile([128, B_LOC], F32, tag="svc")
                    svt = work.tile([128, B_LOC], F32, tag="svt")
                    tmp = work.tile([128, B_LOC], F32, tag="tmp")
                    sls = [slice(0, HB), slice(HB, B_LOC)]
                    # Engines are strict FIFO — emit both halves of each stage
                    # together so the half-chains pipeline, and keep each
                    # half's data in its own PSUM tile (deps are
                    # tile-granular). The time-channel path (vpr/svt/W1Sel8)
                    # is full-width: it has slack and fills the PE gap while
                    # tmp cooks, keeping the PE warm.
                    for h in range(2):
                        nc.scalar.activation(th[:, sls[h]], hpre[h][:],
                                             TANH, bias=b1c[:])
                    for h in range(2):
                        nc.tensor.matmul(vv[h][:], W2aT[:],
                                         th[:, sls[h]], start=True, stop=True)
                    nc.tensor.matmul(vpr[:], W2bRepT[:], th[:],
                                     start=True, stop=True)
                    for h in range(2):
                        nc.scalar.activation(svc[:, sls[h]], vv[h][:],
                                             TANH, bias=b2c[:])
                    nc.scalar.activation(svt[:], vpr[:], TANH, bias=b2tRep[:])
                    for h in range(2):
                        nc.vector.tensor_tensor(tmp[:, sls[h]], svc[:, sls[h]],
                                                dXb[:, sls[h]], MULT)
                    for h in range(2):
                        nc.tensor.matmul(hpre[h][:], W1SelT8[:],
                                         svt[:, sls[h]], start=False,
                                         stop=False, skip_group_check=True)
                    for h in range(2):
                        nc.tensor.matmul(hpre[h][:], W1SelT[:],
                                         tmp[:, sls[h]], start=False,
                                         stop=False, skip_group_check=True)
                # per-interval output: z_{k+1} = pinv(W1) @ hpre (f32r)
                hps = work.tile([128, B_LOC], F32R, tag="hps")
                for h in range(2):
                    nc.vector.tensor_copy(hps[:, h * HB:(h + 1) * HB],
                                          hpre[h][:])
                zt_ps = ps4.tile([16, B_LOC], F32, tag="ztp")
                nc.tensor.matmul(zt_ps[:], RT[:], hps[:], start=True, stop=True)
                zout = zpool.tile([16, B_LOC], F32, tag="z", name=f"zout_{k}")
                nc.vector.tensor_copy(zout[:], zt_ps[:])
                nc.sync.dma_start(d_out.ap()[k], zout[:])
                sprev = srs[k]
                srs.pop(k - 1, None)

    nc.compile()
    _BUILD_CACHE[key] = nc
    return nc


def _prep_core_inputs(us, ys, cst, core, n_intervals):
    b0 = core * B_LOC
    usc = us[:, b0:b0 + B_LOC, :]                       # (L, B, U)
    slope = (usc[1:] - usc[:-1]).transpose(0, 2, 1)     # (L-1, U, B)
    m = {"ys0T": np.ascontiguousarray(ys[0, b0:b0 + B_LOC, :].T).astype(np.float32),
         "srep": np.ascontiguousarray(
             np.repeat(slope[:n_intervals].astype(np.float32), 16, axis=1))}
    m.update(cst)
    return m


def kernel(ts, us, ys, W1, b1, W2, b2, batch_size=None, n_intervals=NI):
    from concourse.bass_utils import run_bass_kernel_spmd

    us = np.asarray(us, dtype=np.float32)
    ys = np.asarray(ys, dtype=np.float32)
    b2arr = np.asarray(b2, np.float32)
    cst = _host_constants(np.asarray(W1, np.float32), np.asarray(b1, np.float32),
                          np.asarray(W2, np.float32), b2arr)
    nc = _build(n_intervals, merged_tanh=bool(np.all(b2arr == 0.0)))
    in_maps = [_prep_core_inputs(us, ys, cst, c, n_intervals) for c in range(N_CORES)]
    res = run_bass_kernel_spmd(nc, in_maps, core_ids=list(range(N_CORES)))
    out = np.empty((B_TOT, n_intervals + 1, Y), dtype=np.float32)
    out[:, 0, :] = ys[0]
    for c in range(N_CORES):
        b0 = c * B_LOC
        out[b0:b0 + B_LOC, 1:, :] = res.results[c]["out"].transpose(2, 0, 1)
    kernel._last_results = res
    return out
